# revision 7
# baseline (speedup 1.0000x reference)
"""Trainium2 Bass kernel for nn_Linear_18494129177115 (moe_routing).

Math (reference, fp32):
  base   = x @ W^T                                  [B,T,O]
  logits = x @ Wr^T + lang_bias                     [B,T,E]
  gates  = scatter(softmax(top2(logits)))           [B,T,E]
  h      = x @ A_e^T  (all experts)                 [B,T,E,R]
  out    = base + SCALING * sum_e gates_e * h_e @ B_e^T

Key identities used here:
- With A_cat = concat_e(A_e) [E*R, D] and B_cat[e*R+r, o] = B[e, o, r],
  the gated LoRA collapses to
    out = x @ W^T + (gates_expanded * (x @ A_cat^T)) @ (SCALING * B_cat),
  two thin matmuls fused into the base GEMM's PSUM accumulation.
- All heavy GEMMs run as bf16 hi/lo 3-pass splits (x ~ xh + xl,
  W ~ Wh + Wl, x@W ~ xh@Wh + xl@Wh + xh@Wl): bf16 matmuls stream 2
  cols/cycle with fast weight load on TRN2, so 3 passes beat one
  fp32/fp32r pass ~3.4x while keeping ~1e-5 relative accuracy
  (bf16 products are exact in fp32; only the xl@Wl term is dropped).
- Router logits use true fp32 matmuls on a streamed fp32 copy of x so
  the top-2 selection stays faithful to the fp32 reference.

Sharding: data-parallel over tokens, 1024 tokens/core on 8 cores; all
weights replicated; no collectives. Each core's tokens lie in a single
batch row, so the language bias is a per-core constant row (tiny input).

Per-core layout: tokens on PSUM partitions, D contracted in 32 chunks of
128. x^T (hi+lo bf16) is SBUF-resident; W^T (hi+lo bf16) streams once.
"""

import numpy as np

LANG_BIAS = 5.0
SCALING = 32.0 / 16.0
B_SZ, T_SZ, D_SZ, O_SZ, E_SZ, R_SZ = 4, 2048, 4096, 4096, 8, 16
NCORES = 8
TPC = (B_SZ * T_SZ) // NCORES      # 1024 tokens per core
NT = TPC // 128                    # 8 token tiles per core
NK = D_SZ // 128                   # 32 contraction chunks
NO = O_SZ // 512                   # 8 output tiles of 512
ER = E_SZ * R_SZ                   # 128 (expert, rank) pairs

_CACHE: dict = {}
LAST_RESULT = None


def _build_bass(loop_n: int | None = None):
    import concourse.bacc as bacc
    import concourse.mybir as mybir
    from concourse import tile
    from concourse.masks import make_identity

    f32 = mybir.dt.float32
    bf16 = mybir.dt.bfloat16
    AX = mybir.AxisListType.X
    OP = mybir.AluOpType
    ACT = mybir.ActivationFunctionType

    nc = bacc.Bacc(None, target_bir_lowering=False, debug=False)

    xt_d = nc.dram_tensor("xt", [2, NK, 128, TPC], bf16, kind="ExternalInput")
    xr_d = nc.dram_tensor("xr", [NK, 128, TPC], f32, kind="ExternalInput")
    wt_d = nc.dram_tensor("wt", [NK, NO, 2, 128, 512], bf16, kind="ExternalInput")
    acat_d = nc.dram_tensor("acat", [2, NK, 128, ER], bf16, kind="ExternalInput")
    wrt_d = nc.dram_tensor("wrt", [NK, 128, E_SZ], f32, kind="ExternalInput")
    bcat_d = nc.dram_tensor("bcat", [2, ER, O_SZ], bf16, kind="ExternalInput")
    bias_d = nc.dram_tensor("biasx", [128, E_SZ], f32, kind="ExternalInput")
    sel_d = nc.dram_tensor("sel", [E_SZ, ER], f32, kind="ExternalInput")
    out_d = nc.dram_tensor("out", [NT, 128, NO, 512], f32, kind="ExternalOutput")

    with tile.TileContext(nc) as tc:
        with (
            tc.tile_pool(name="const", bufs=1) as cpool,
            tc.tile_pool(name="wstream", bufs=4) as wpool,
            tc.tile_pool(name="rstream", bufs=2) as rpool,
            tc.tile_pool(name="ostage", bufs=2) as opool,
            tc.tile_pool(name="gate", bufs=2) as gpool,
            tc.tile_pool(name="psum", bufs=8, space="PSUM") as psum,
        ):

            def body(_iv=None):
                # ---- resident inputs ----
                xh_sb = cpool.tile([128, NK, TPC], bf16, name="xh_sb")
                xl_sb = cpool.tile([128, NK, TPC], bf16, name="xl_sb")
                ach_sb = cpool.tile([128, NK, ER], bf16, name="ach_sb")
                acl_sb = cpool.tile([128, NK, ER], bf16, name="acl_sb")
                wrt_sb = cpool.tile([128, NK, E_SZ], f32, name="wrt_sb")
                bch_sb = cpool.tile([ER, O_SZ], bf16, name="bch_sb")
                bcl_sb = cpool.tile([ER, O_SZ], bf16, name="bcl_sb")
                bias_sb = cpool.tile([128, E_SZ], f32, name="bias_sb")
                sel_sb = cpool.tile([E_SZ, ER], f32, name="sel_sb")
                ident_sb = cpool.tile([128, 128], f32, name="ident_sb")
                hT_sb = cpool.tile([128, TPC], f32, name="hT_sb")
                ghh_sb = cpool.tile([128, NT, 128], bf16, name="ghh_sb")
                ghl_sb = cpool.tile([128, NT, 128], bf16, name="ghl_sb")

                for kc in range(NK):
                    nc.sync.dma_start(xh_sb[:, kc, :], xt_d[0, kc])
                    nc.sync.dma_start(xl_sb[:, kc, :], xt_d[1, kc])
                nc.sync.dma_start(ach_sb[:], acat_d[0].rearrange("k p e -> p k e"))
                nc.sync.dma_start(acl_sb[:], acat_d[1].rearrange("k p e -> p k e"))
                nc.sync.dma_start(wrt_sb[:], wrt_d[:].rearrange("k p e -> p k e"))
                nc.sync.dma_start(bch_sb[:], bcat_d[0])
                nc.sync.dma_start(bcl_sb[:], bcat_d[1])
                nc.sync.dma_start(bias_sb[:], bias_d[:])
                nc.sync.dma_start(sel_sb[:], sel_d[:])
                make_identity(nc, ident_sb[:])

                # ---- phase 1a: router logits (fp32) on streamed fp32 x ----
                pls = [
                    psum.tile([128, E_SZ], f32, tag="bank", name=f"pl{t}")
                    for t in range(NT)
                ]
                for kc in range(NK):
                    xr_t = rpool.tile([128, TPC], f32, name="xr_t")
                    nc.sync.dma_start(xr_t[:], xr_d[kc])
                    for tt in range(NT):
                        nc.tensor.matmul(
                            pls[tt][:],
                            xr_t[:, tt * 128 : (tt + 1) * 128],
                            wrt_sb[:, kc, :],
                            start=(kc == 0),
                            stop=(kc == NK - 1),
                        )

                # ---- phase 1b: h^T = A_cat @ x^T  (bf16 x3) ----
                for tb in range(TPC // 512):
                    ph = psum.tile([128, 512], f32, tag="bank", name=f"ph{tb}")
                    sl = slice(tb * 512, (tb + 1) * 512)
                    for kc in range(NK):
                        first = kc == 0
                        last = kc == NK - 1
                        nc.tensor.matmul(
                            ph[:], ach_sb[:, kc, :], xh_sb[:, kc, sl],
                            start=first, stop=False,
                        )
                        nc.tensor.matmul(
                            ph[:], ach_sb[:, kc, :], xl_sb[:, kc, sl],
                            start=False, stop=False,
                        )
                        nc.tensor.matmul(
                            ph[:], acl_sb[:, kc, :], xh_sb[:, kc, sl],
                            start=False, stop=last,
                        )
                    nc.vector.tensor_copy(hT_sb[:, sl], ph[:])

                # ---- phase 1c: top-2 softmax gates + gh^T (hi/lo) ----
                for tt in range(NT):
                    ts = slice(tt * 128, (tt + 1) * 128)
                    logit = gpool.tile([128, E_SZ], f32, name="logit")
                    nc.vector.tensor_tensor(
                        logit[:], pls[tt][:], bias_sb[:], op=OP.add
                    )
                    m1 = gpool.tile([128, 1], f32, name="m1")
                    nc.vector.reduce_max(m1[:], logit[:], axis=AX)
                    mask1 = gpool.tile([128, E_SZ], f32, name="mask1")
                    nc.vector.tensor_scalar(
                        mask1[:], logit[:], m1[:], None, op0=OP.is_equal
                    )
                    l2 = gpool.tile([128, E_SZ], f32, name="l2")
                    nc.vector.tensor_scalar(l2[:], mask1[:], -1e30, None, op0=OP.mult)
                    nc.vector.tensor_tensor(l2[:], l2[:], logit[:], op=OP.add)
                    m2 = gpool.tile([128, 1], f32, name="m2")
                    nc.vector.reduce_max(m2[:], l2[:], axis=AX)
                    mask2 = gpool.tile([128, E_SZ], f32, name="mask2")
                    nc.vector.tensor_scalar(
                        mask2[:], l2[:], m2[:], None, op0=OP.is_equal
                    )
                    w1 = gpool.tile([128, 1], f32, name="w1")
                    nc.scalar.activation(
                        w1[:], m2[:], ACT.Sigmoid, bias=m1[:], scale=-1.0
                    )
                    w2 = gpool.tile([128, 1], f32, name="w2")
                    nc.vector.tensor_scalar(
                        w2[:], w1[:], -1.0, 1.0, op0=OP.mult, op1=OP.add
                    )
                    g1 = gpool.tile([128, E_SZ], f32, name="g1")
                    nc.vector.tensor_scalar(g1[:], mask1[:], w1[:], None, op0=OP.mult)
                    gates = gpool.tile([128, E_SZ], f32, name="gates")
                    nc.vector.tensor_scalar(
                        gates[:], mask2[:], w2[:], None, op0=OP.mult
                    )
                    nc.vector.tensor_tensor(gates[:], gates[:], g1[:], op=OP.add)

                    # gates [t,E] -> [E,t] -> expand to [ER,t]; gh^T = gexp * h^T
                    ptr = psum.tile([E_SZ, 128], f32, tag="bank", name=f"ptr{tt}")
                    nc.tensor.transpose(ptr[:], gates[:], ident_sb[:])
                    gT = gpool.tile([E_SZ, 128], f32, name="gT")
                    nc.vector.tensor_copy(gT[:], ptr[:])
                    pge = psum.tile([128, 128], f32, tag="bank", name=f"pge{tt}")
                    nc.tensor.matmul(pge[:], sel_sb[:], gT[:], start=True, stop=True)
                    gh32 = gpool.tile([128, 128], f32, name="gh32")
                    nc.vector.tensor_tensor(gh32[:], pge[:], hT_sb[:, ts], op=OP.mult)
                    nc.vector.tensor_copy(ghh_sb[:, tt, :], gh32[:])
                    glo = gpool.tile([128, 128], f32, name="glo")
                    nc.vector.tensor_tensor(
                        glo[:], gh32[:], ghh_sb[:, tt, :], op=OP.subtract
                    )
                    nc.vector.tensor_copy(ghl_sb[:, tt, :], glo[:])

                # ---- phase 2: out = x @ W^T (+ gh @ SCALING*B_cat), bf16 x3 ----
                for ot in range(NO):
                    po = [
                        psum.tile([128, 512], f32, tag="bank", name=f"po{ot}_{i}")
                        for i in range(NT)
                    ]
                    for kc in range(NK):
                        w_t = wpool.tile([128, 2, 512], bf16, name="w_t")
                        nc.sync.dma_start(
                            w_t[:], wt_d[kc, ot].rearrange("two p n -> p two n")
                        )
                        wh = w_t[:, 0, :]
                        wl = w_t[:, 1, :]
                        for tt in range(NT):
                            xh = xh_sb[:, kc, tt * 128 : (tt + 1) * 128]
                            xl = xl_sb[:, kc, tt * 128 : (tt + 1) * 128]
                            nc.tensor.matmul(
                                po[tt][:], xh, wh, start=(kc == 0), stop=False
                            )
                            nc.tensor.matmul(po[tt][:], xh, wl, start=False, stop=False)
                            nc.tensor.matmul(po[tt][:], xl, wh, start=False, stop=False)
                    osl = slice(ot * 512, (ot + 1) * 512)
                    for tt in range(NT):
                        ghh = ghh_sb[:, tt, :]
                        ghl = ghl_sb[:, tt, :]
                        nc.tensor.matmul(
                            po[tt][:], ghh, bch_sb[:, osl], start=False, stop=False
                        )
                        nc.tensor.matmul(
                            po[tt][:], ghh, bcl_sb[:, osl], start=False, stop=False
                        )
                        nc.tensor.matmul(
                            po[tt][:], ghl, bch_sb[:, osl], start=False, stop=True
                        )
                        ob = opool.tile([128, 512], f32, name="ob")
                        nc.vector.tensor_copy(ob[:], po[tt][:])
                        nc.sync.dma_start(out_d[tt, :, ot, :], ob[:])

            if loop_n is None:
                body()
            else:
                with tc.For_i(0, loop_n, 1) as iv:
                    body(iv)

    nc.compile()
    return nc


def _split_bf16(a):
    import ml_dtypes

    hi = a.astype(ml_dtypes.bfloat16)
    lo = (a - hi.astype(np.float32)).astype(ml_dtypes.bfloat16)
    return hi, lo


def _host_prep(x, language_ids, W, Wr, A, B):
    import ml_dtypes

    x = np.asarray(x, dtype=np.float32)
    W = np.asarray(W, dtype=np.float32)
    Wr = np.asarray(Wr, dtype=np.float32)
    A = np.asarray(A, dtype=np.float32)
    B = np.asarray(B, dtype=np.float32)
    lang = np.asarray(language_ids).astype(np.int64)

    xf = np.ascontiguousarray(x.reshape(B_SZ * T_SZ, D_SZ))
    # W^T [D,O] -> [kc, ot, 2(hi/lo), 128, 512] contiguous DMA blocks
    wtT = W.T.reshape(NK, 128, NO, 512).transpose(0, 2, 1, 3)  # [kc, ot, 128, 512]
    wh, wl = _split_bf16(wtT)
    wt = np.ascontiguousarray(np.stack([wh, wl], axis=2))      # [NK, NO, 2, 128, 512]

    acat_t = np.ascontiguousarray(A.reshape(ER, D_SZ).T).reshape(NK, 128, ER)
    ah, al = _split_bf16(acat_t)
    acat = np.ascontiguousarray(np.stack([ah, al], axis=0))    # [2, NK, 128, ER]

    wrt = np.ascontiguousarray(Wr.T).reshape(NK, 128, E_SZ)

    bcat32 = (SCALING * B.transpose(0, 2, 1)).reshape(ER, O_SZ)
    bh, bl = _split_bf16(bcat32)
    bcat = np.ascontiguousarray(np.stack([bh, bl], axis=0))    # [2, ER, O]

    sel = np.zeros((E_SZ, ER), dtype=np.float32)
    sel[np.arange(ER) // R_SZ, np.arange(ER)] = 1.0

    in_maps = []
    for c in range(NCORES):
        shard = xf[c * TPC : (c + 1) * TPC]
        xr = np.ascontiguousarray(shard.T).reshape(NK, 128, TPC)
        xhh, xll = _split_bf16(xr)
        xt = np.ascontiguousarray(np.stack([xhh, xll], axis=0))  # [2, NK, 128, TPC]
        b = int(lang[(c * TPC) // T_SZ])
        brow = np.zeros(E_SZ, dtype=np.float32)
        if b >= 0:
            brow[b] = LANG_BIAS
        bias = np.ascontiguousarray(np.tile(brow, (128, 1)))
        in_maps.append(
            {
                "xt": xt,
                "xr": xr,
                "wt": wt,
                "acat": acat,
                "wrt": wrt,
                "bcat": bcat,
                "biasx": bias,
                "sel": sel,
            }
        )
    return in_maps


def kernel(x, language_ids, W, Wr, A, B):
    global LAST_RESULT
    from concourse.bass_utils import run_bass_kernel_spmd

    if "nc" not in _CACHE:
        _CACHE["nc"] = _build_bass()
    nc = _CACHE["nc"]

    in_maps = _host_prep(x, language_ids, W, Wr, A, B)
    res = run_bass_kernel_spmd(nc, in_maps, core_ids=list(range(NCORES)))
    LAST_RESULT = res
    outs = [r["out"].reshape(TPC, O_SZ) for r in res.results]
    return np.concatenate(outs, axis=0).reshape(B_SZ, T_SZ, O_SZ)


# revision 8
# speedup vs baseline: 1.4999x; 1.4999x over previous
"""Trainium2 Bass kernel for nn_Linear_18494129177115 (moe_routing).

Math (reference, fp32):
  base   = x @ W^T                                  [B,T,O]
  logits = x @ Wr^T + lang_bias                     [B,T,E]
  gates  = scatter(softmax(top2(logits)))           [B,T,E]
  h      = x @ A_e^T  (all experts)                 [B,T,E,R]
  out    = base + SCALING * sum_e gates_e * h_e @ B_e^T

Key design points:
- With A_cat = concat_e(A_e) [E*R, D] and B_cat[e*R+r, o] = B[e, o, r],
  the gated LoRA collapses to
    out = x @ W^T + (gates_expanded * (x @ A_cat^T)) @ (SCALING * B_cat),
  two thin matmuls fused into the base GEMM's PSUM accumulation.
- Heavy GEMMs run as bf16 hi/lo 3-pass splits (x@W ~ xh@Wh + xl@Wh +
  xh@Wl): bf16 products are exact in fp32, only the lo*lo term is
  dropped (~1e-5 rel error), and bf16 matmuls are ~4x faster than
  fp32/fp32r on the PE.
- Router logits use true fp32 matmuls on a streamed fp32 copy of x so
  top-2 selection stays faithful to the fp32 reference.
- DMA: batched into >=512KB partition-row-contiguous transfers, spread
  over the two HWDGE rings (sync, scalar) + the SWDGE ring (gpsimd) --
  per-dma fixed cost is ~2us and each ring is FIFO, so few big DMAs on
  three rings instead of many small ones on one.

Sharding: data-parallel over tokens, 1024 tokens/core on 8 cores; all
weights replicated; no collectives. Each core's tokens lie in a single
batch row, so the language bias is a per-core constant row (tiny input).
"""

import numpy as np

LANG_BIAS = 5.0
SCALING = 32.0 / 16.0
B_SZ, T_SZ, D_SZ, O_SZ, E_SZ, R_SZ = 4, 2048, 4096, 4096, 8, 16
NCORES = 8
TPC = (B_SZ * T_SZ) // NCORES      # 1024 tokens per core
NT = TPC // 128                    # 8 token tiles per core
NK = D_SZ // 128                   # 32 contraction chunks
NKG = NK // 2                      # 16 W-stream chunks (2 kc each)
NO = O_SZ // 512                   # 8 output tiles of 512
ER = E_SZ * R_SZ                   # 128 (expert, rank) pairs

_CACHE: dict = {}
LAST_RESULT = None


def _build_bass(loop_n: int | None = None):
    import concourse.bacc as bacc
    import concourse.mybir as mybir
    from concourse import tile
    from concourse.masks import make_identity

    f32 = mybir.dt.float32
    bf16 = mybir.dt.bfloat16
    AX = mybir.AxisListType.X
    OP = mybir.AluOpType
    ACT = mybir.ActivationFunctionType

    nc = bacc.Bacc(None, target_bir_lowering=False, debug=False)

    # [2(hi/lo), kc, p, t]
    xt_d = nc.dram_tensor("xt", [2, NK, 128, TPC], bf16, kind="ExternalInput")
    # fp32 x for the router
    xr_d = nc.dram_tensor("xr", [NK, 128, TPC], f32, kind="ExternalInput")
    # W^T stream: per (ot, kcg) a [128, 2kc, 2hl, 512] block, rows contiguous
    wt_d = nc.dram_tensor(
        "wt", [NO, NKG, 128, 2, 2, 512], bf16, kind="ExternalInput"
    )
    # A_cat^T stream: per kc a [128, 2hl, ER] block
    acat_d = nc.dram_tensor("acat", [NK, 128, 2, ER], bf16, kind="ExternalInput")
    wrt_d = nc.dram_tensor("wrt", [NK, 128, E_SZ], f32, kind="ExternalInput")
    bcat_d = nc.dram_tensor("bcat", [2, ER, O_SZ], bf16, kind="ExternalInput")
    bias_d = nc.dram_tensor("biasx", [128, E_SZ], f32, kind="ExternalInput")
    sel_d = nc.dram_tensor("sel", [E_SZ, ER], f32, kind="ExternalInput")
    out_d = nc.dram_tensor("out", [NO, NT, 128, 512], f32, kind="ExternalOutput")

    with tile.TileContext(nc) as tc:
        with (
            tc.tile_pool(name="const", bufs=1) as cpool,
            tc.tile_pool(name="wstream", bufs=3) as wpool,
            tc.tile_pool(name="acstream", bufs=3) as acpool,
            tc.tile_pool(name="rstream", bufs=2) as rpool,
            tc.tile_pool(name="ostage", bufs=2) as opool,
            tc.tile_pool(name="gate", bufs=2) as gpool,
            tc.tile_pool(name="psum", bufs=8, space="PSUM") as psum,
        ):

            def body(_iv=None):
                # ---- resident inputs ----
                xh_sb = cpool.tile([128, NK, TPC], bf16, name="xh_sb")
                xl_sb = cpool.tile([128, NK, TPC], bf16, name="xl_sb")
                wrt_sb = cpool.tile([128, NK, E_SZ], f32, name="wrt_sb")
                bch_sb = cpool.tile([ER, O_SZ], bf16, name="bch_sb")
                bcl_sb = cpool.tile([ER, O_SZ], bf16, name="bcl_sb")
                bias_sb = cpool.tile([128, E_SZ], f32, name="bias_sb")
                sel_sb = cpool.tile([E_SZ, ER], f32, name="sel_sb")
                ident_sb = cpool.tile([128, 128], f32, name="ident_sb")
                hT_sb = cpool.tile([128, TPC], f32, name="hT_sb")
                ghh_sb = cpool.tile([128, NT, 128], bf16, name="ghh_sb")
                ghl_sb = cpool.tile([128, NT, 128], bf16, name="ghl_sb")

                # x hi/lo: 1MB per DMA, alternating HWDGE rings
                for g in range(NK // 4):
                    ksl = slice(g * 4, (g + 1) * 4)
                    eng = nc.sync if g % 2 == 0 else nc.scalar
                    eng.dma_start(
                        xh_sb[:, ksl, :], xt_d[0, ksl].rearrange("k p t -> p k t")
                    )
                    eng = nc.scalar if g % 2 == 0 else nc.sync
                    eng.dma_start(
                        xl_sb[:, ksl, :], xt_d[1, ksl].rearrange("k p t -> p k t")
                    )
                nc.scalar.dma_start(wrt_sb[:], wrt_d[:].rearrange("k p e -> p k e"))
                nc.scalar.dma_start(bch_sb[:], bcat_d[0])
                nc.scalar.dma_start(bcl_sb[:], bcat_d[1])
                nc.scalar.dma_start(bias_sb[:], bias_d[:])
                nc.scalar.dma_start(sel_sb[:], sel_d[:])
                make_identity(nc, ident_sb[:])

                # ---- phase 1a: router logits (fp32) on streamed fp32 x ----
                pls = [
                    psum.tile([128, E_SZ], f32, tag="bank", name=f"pl{t}")
                    for t in range(NT)
                ]
                for kc in range(NK):
                    xr_t = rpool.tile([128, TPC], f32, name="xr_t")
                    nc.gpsimd.dma_start(xr_t[:], xr_d[kc])
                    for tt in range(NT):
                        nc.tensor.matmul(
                            pls[tt][:],
                            xr_t[:, tt * 128 : (tt + 1) * 128],
                            wrt_sb[:, kc, :],
                            start=(kc == 0),
                            stop=(kc == NK - 1),
                        )

                # ---- phase 1b: h^T = A_cat @ x^T  (bf16 x3), acat streamed ----
                ph = [
                    psum.tile([128, 512], f32, tag="bank", name=f"ph{t}")
                    for t in range(TPC // 512)
                ]
                for kc in range(NK):
                    ac_t = acpool.tile([128, 2, ER], bf16, name="ac_t")
                    nc.scalar.dma_start(ac_t[:], acat_d[kc])
                    ach = ac_t[:, 0, :]
                    acl = ac_t[:, 1, :]
                    first = kc == 0
                    last = kc == NK - 1
                    for tb in range(TPC // 512):
                        sl = slice(tb * 512, (tb + 1) * 512)
                        nc.tensor.matmul(
                            ph[tb][:], ach, xh_sb[:, kc, sl], start=first, stop=False
                        )
                        nc.tensor.matmul(
                            ph[tb][:], ach, xl_sb[:, kc, sl], start=False, stop=False
                        )
                        nc.tensor.matmul(
                            ph[tb][:], acl, xh_sb[:, kc, sl], start=False, stop=last
                        )
                for tb in range(TPC // 512):
                    nc.vector.tensor_copy(
                        hT_sb[:, tb * 512 : (tb + 1) * 512], ph[tb][:]
                    )

                # ---- phase 1c: top-2 softmax gates + gh^T (hi/lo) ----
                for tt in range(NT):
                    ts = slice(tt * 128, (tt + 1) * 128)
                    logit = gpool.tile([128, E_SZ], f32, name="logit")
                    nc.vector.tensor_tensor(
                        logit[:], pls[tt][:], bias_sb[:], op=OP.add
                    )
                    m1 = gpool.tile([128, 1], f32, name="m1")
                    nc.vector.reduce_max(m1[:], logit[:], axis=AX)
                    mask1 = gpool.tile([128, E_SZ], f32, name="mask1")
                    nc.vector.tensor_scalar(
                        mask1[:], logit[:], m1[:], None, op0=OP.is_equal
                    )
                    l2 = gpool.tile([128, E_SZ], f32, name="l2")
                    nc.vector.tensor_scalar(l2[:], mask1[:], -1e30, None, op0=OP.mult)
                    nc.vector.tensor_tensor(l2[:], l2[:], logit[:], op=OP.add)
                    m2 = gpool.tile([128, 1], f32, name="m2")
                    nc.vector.reduce_max(m2[:], l2[:], axis=AX)
                    mask2 = gpool.tile([128, E_SZ], f32, name="mask2")
                    nc.vector.tensor_scalar(
                        mask2[:], l2[:], m2[:], None, op0=OP.is_equal
                    )
                    w1 = gpool.tile([128, 1], f32, name="w1")
                    nc.scalar.activation(
                        w1[:], m2[:], ACT.Sigmoid, bias=m1[:], scale=-1.0
                    )
                    w2 = gpool.tile([128, 1], f32, name="w2")
                    nc.vector.tensor_scalar(
                        w2[:], w1[:], -1.0, 1.0, op0=OP.mult, op1=OP.add
                    )
                    g1 = gpool.tile([128, E_SZ], f32, name="g1")
                    nc.vector.tensor_scalar(g1[:], mask1[:], w1[:], None, op0=OP.mult)
                    gates = gpool.tile([128, E_SZ], f32, name="gates")
                    nc.vector.tensor_scalar(
                        gates[:], mask2[:], w2[:], None, op0=OP.mult
                    )
                    nc.vector.tensor_tensor(gates[:], gates[:], g1[:], op=OP.add)

                    ptr = psum.tile([E_SZ, 128], f32, tag="bank", name=f"ptr{tt}")
                    nc.tensor.transpose(ptr[:], gates[:], ident_sb[:])
                    gT = gpool.tile([E_SZ, 128], f32, name="gT")
                    nc.vector.tensor_copy(gT[:], ptr[:])
                    pge = psum.tile([128, 128], f32, tag="bank", name=f"pge{tt}")
                    nc.tensor.matmul(pge[:], sel_sb[:], gT[:], start=True, stop=True)
                    gh32 = gpool.tile([128, 128], f32, name="gh32")
                    nc.vector.tensor_tensor(gh32[:], pge[:], hT_sb[:, ts], op=OP.mult)
                    nc.vector.tensor_copy(ghh_sb[:, tt, :], gh32[:])
                    glo = gpool.tile([128, 128], f32, name="glo")
                    nc.vector.tensor_tensor(
                        glo[:], gh32[:], ghh_sb[:, tt, :], op=OP.subtract
                    )
                    nc.vector.tensor_copy(ghl_sb[:, tt, :], glo[:])

                # ---- phase 2: out = x @ W^T (+ gh @ SCALING*B_cat), bf16 x3 ----
                for ot in range(NO):
                    po = [
                        psum.tile([128, 512], f32, tag="bank", name=f"po{ot}_{i}")
                        for i in range(NT)
                    ]
                    for kcg in range(NKG):
                        w_t = wpool.tile([128, 2, 2, 512], bf16, name="w_t")
                        eng = nc.sync if kcg % 2 == 0 else nc.scalar
                        eng.dma_start(w_t[:], wt_d[ot, kcg])
                        for g in range(2):
                            kc = kcg * 2 + g
                            wh = w_t[:, g, 0, :]
                            wl = w_t[:, g, 1, :]
                            for tt in range(NT):
                                xh = xh_sb[:, kc, tt * 128 : (tt + 1) * 128]
                                xl = xl_sb[:, kc, tt * 128 : (tt + 1) * 128]
                                nc.tensor.matmul(
                                    po[tt][:], xh, wh, start=(kc == 0), stop=False
                                )
                                nc.tensor.matmul(
                                    po[tt][:], xh, wl, start=False, stop=False
                                )
                                nc.tensor.matmul(
                                    po[tt][:], xl, wh, start=False, stop=False
                                )
                    osl = slice(ot * 512, (ot + 1) * 512)
                    for tt in range(NT):
                        ghh = ghh_sb[:, tt, :]
                        ghl = ghl_sb[:, tt, :]
                        nc.tensor.matmul(
                            po[tt][:], ghh, bch_sb[:, osl], start=False, stop=False
                        )
                        nc.tensor.matmul(
                            po[tt][:], ghh, bcl_sb[:, osl], start=False, stop=False
                        )
                        nc.tensor.matmul(
                            po[tt][:], ghl, bch_sb[:, osl], start=False, stop=True
                        )
                        ob = opool.tile([128, 512], f32, name="ob")
                        nc.vector.tensor_copy(ob[:], po[tt][:])
                        eng = nc.gpsimd if tt % 2 == 0 else nc.scalar
                        eng.dma_start(out_d[ot, tt], ob[:])

            if loop_n is None:
                body()
            else:
                with tc.For_i(0, loop_n, 1) as iv:
                    body(iv)

    nc.compile()
    return nc


def _split_bf16(a):
    import ml_dtypes

    hi = a.astype(ml_dtypes.bfloat16)
    lo = (a - hi.astype(np.float32)).astype(ml_dtypes.bfloat16)
    return hi, lo


def _host_prep(x, language_ids, W, Wr, A, B):
    x = np.asarray(x, dtype=np.float32)
    W = np.asarray(W, dtype=np.float32)
    Wr = np.asarray(Wr, dtype=np.float32)
    A = np.asarray(A, dtype=np.float32)
    B = np.asarray(B, dtype=np.float32)
    lang = np.asarray(language_ids).astype(np.int64)

    xf = np.ascontiguousarray(x.reshape(B_SZ * T_SZ, D_SZ))

    # W^T [D,O]: hi/lo split, then [NO, NKG, 128, 2kc, 2hl, 512] row-contiguous
    wtT = W.T.reshape(NK, 128, NO, 512)                  # [kc, p, ot, n]
    wh, wl = _split_bf16(wtT)
    warr = np.stack([wh, wl], axis=-2)                   # [kc, p, ot, 2hl, n]
    warr = warr.reshape(NKG, 2, 128, NO, 2, 512)         # [kcg, g, p, ot, hl, n]
    wt = np.ascontiguousarray(warr.transpose(3, 0, 2, 1, 4, 5))

    acat_t = np.ascontiguousarray(A.reshape(ER, D_SZ).T).reshape(NK, 128, ER)
    ah, al = _split_bf16(acat_t)
    acat = np.ascontiguousarray(np.stack([ah, al], axis=2))  # [kc, p, 2, ER]

    wrt = np.ascontiguousarray(Wr.T).reshape(NK, 128, E_SZ)

    bcat32 = (SCALING * B.transpose(0, 2, 1)).reshape(ER, O_SZ)
    bh, bl = _split_bf16(bcat32)
    bcat = np.ascontiguousarray(np.stack([bh, bl], axis=0))  # [2, ER, O]

    sel = np.zeros((E_SZ, ER), dtype=np.float32)
    sel[np.arange(ER) // R_SZ, np.arange(ER)] = 1.0

    in_maps = []
    for c in range(NCORES):
        shard = xf[c * TPC : (c + 1) * TPC]
        xr = np.ascontiguousarray(shard.T).reshape(NK, 128, TPC)
        xhh, xll = _split_bf16(xr)
        xt = np.ascontiguousarray(np.stack([xhh, xll], axis=0))  # [2, kc, p, t]
        b = int(lang[(c * TPC) // T_SZ])
        brow = np.zeros(E_SZ, dtype=np.float32)
        if b >= 0:
            brow[b] = LANG_BIAS
        bias = np.ascontiguousarray(np.tile(brow, (128, 1)))
        in_maps.append(
            {
                "xt": xt,
                "xr": xr,
                "wt": wt,
                "acat": acat,
                "wrt": wrt,
                "bcat": bcat,
                "biasx": bias,
                "sel": sel,
            }
        )
    return in_maps


def kernel(x, language_ids, W, Wr, A, B):
    global LAST_RESULT
    from concourse.bass_utils import run_bass_kernel_spmd

    if "nc" not in _CACHE:
        _CACHE["nc"] = _build_bass()
    nc = _CACHE["nc"]

    in_maps = _host_prep(x, language_ids, W, Wr, A, B)
    res = run_bass_kernel_spmd(nc, in_maps, core_ids=list(range(NCORES)))
    LAST_RESULT = res
    outs = [
        r["out"].transpose(1, 2, 0, 3).reshape(TPC, O_SZ) for r in res.results
    ]
    return np.concatenate(outs, axis=0).reshape(B_SZ, T_SZ, O_SZ)


# revision 10
# speedup vs baseline: 5.6706x; 3.7807x over previous
"""Trainium2 Bass kernel for nn_Linear_18494129177115 (moe_routing).

Math (reference, fp32):
  base   = x @ W^T                                  [B,T,O]
  logits = x @ Wr^T + lang_bias                     [B,T,E]
  gates  = scatter(softmax(top2(logits)))           [B,T,E]
  h      = x @ A_e^T  (all experts)                 [B,T,E,R]
  out    = base + SCALING * sum_e gates_e * h_e @ B_e^T

Key design points:
- With A_cat = concat_e(A_e) [E*R, D] and B_cat[e*R+r, o] = B[e, o, r],
  the gated LoRA collapses to
    out = x @ W^T + (gates_expanded * (x @ A_cat^T)) @ (SCALING * B_cat),
  two thin matmuls fused into the base GEMM's PSUM accumulation.
- Heavy GEMMs run as bf16 hi/lo 3-pass splits (x@W ~ xh@Wh + xl@Wh +
  xh@Wl): bf16 products are exact in fp32, only the lo*lo term is
  dropped (~1e-5 rel error), and bf16 matmuls are ~4x faster than
  fp32/fp32r on the PE.
- Router logits use true fp32 matmuls on a streamed fp32 copy of x so
  top-2 selection stays faithful to the fp32 reference.
- DMA: batched into >=512KB partition-row-contiguous transfers, spread
  over the two HWDGE rings (sync, scalar) + the SWDGE ring (gpsimd) --
  per-dma fixed cost is ~2us and each ring is FIFO, so few big DMAs on
  three rings instead of many small ones on one.

Sharding: data-parallel over tokens, 1024 tokens/core on 8 cores; all
weights replicated; no collectives. Each core's tokens lie in a single
batch row, so the language bias is a per-core constant row (tiny input).
"""

import numpy as np

LANG_BIAS = 5.0
SCALING = 32.0 / 16.0
B_SZ, T_SZ, D_SZ, O_SZ, E_SZ, R_SZ = 4, 2048, 4096, 4096, 8, 16
NCORES = 8
TPC = (B_SZ * T_SZ) // NCORES      # 1024 tokens per core
NT = TPC // 128                    # 8 token tiles per core
NK = D_SZ // 128                   # 32 contraction chunks
NKG = NK // 2                      # 16 W-stream chunks (2 kc each)
NO = O_SZ // 512                   # 8 output tiles of 512
ER = E_SZ * R_SZ                   # 128 (expert, rank) pairs

_CACHE: dict = {}
LAST_RESULT = None


def _build_bass(loop_n: int | None = None, parts=("router", "h", "gate", "p2")):
    import concourse.bacc as bacc
    import concourse.mybir as mybir
    from concourse import tile
    from concourse.masks import make_identity

    f32 = mybir.dt.float32
    bf16 = mybir.dt.bfloat16
    AX = mybir.AxisListType.X
    OP = mybir.AluOpType
    ACT = mybir.ActivationFunctionType

    nc = bacc.Bacc(None, target_bir_lowering=False, debug=False)

    # [2(hi/lo), kc, p, t]
    xt_d = nc.dram_tensor("xt", [2, NK, 128, TPC], bf16, kind="ExternalInput")
    # fp32 x for the router
    xr_d = nc.dram_tensor("xr", [NK, 128, TPC], f32, kind="ExternalInput")
    # W^T stream: per (ot, kcg) a [128, 2kc, 2hl, 512] block, rows contiguous
    wt_d = nc.dram_tensor(
        "wt", [NO, NKG, 128, 2, 2, 512], bf16, kind="ExternalInput"
    )
    # A_cat^T stream: per kc a [128, 2hl, ER] block
    acat_d = nc.dram_tensor("acat", [NK, 128, 2, ER], bf16, kind="ExternalInput")
    wrt_d = nc.dram_tensor("wrt", [NK, 128, E_SZ], f32, kind="ExternalInput")
    bcat_d = nc.dram_tensor("bcat", [2, ER, O_SZ], bf16, kind="ExternalInput")
    bias_d = nc.dram_tensor("biasx", [128, E_SZ], f32, kind="ExternalInput")
    sel_d = nc.dram_tensor("sel", [E_SZ, ER], f32, kind="ExternalInput")
    out_d = nc.dram_tensor("out", [NO, NT, 128, 512], f32, kind="ExternalOutput")

    with tile.TileContext(nc) as tc:
        with (
            tc.tile_pool(name="const", bufs=1) as cpool,
            tc.tile_pool(name="wstream", bufs=3) as wpool,
            tc.tile_pool(name="acstream", bufs=3) as acpool,
            tc.tile_pool(name="rstream", bufs=2) as rpool,
            tc.tile_pool(name="ostage", bufs=2) as opool,
            tc.tile_pool(name="gate", bufs=2) as gpool,
            tc.tile_pool(name="psum", bufs=8, space="PSUM") as psum,
        ):

            def body(_iv=None):
                # ---- resident inputs ----
                xh_sb = cpool.tile([128, NK, TPC], bf16, name="xh_sb")
                xl_sb = cpool.tile([128, NK, TPC], bf16, name="xl_sb")
                wrt_sb = cpool.tile([128, NK, E_SZ], f32, name="wrt_sb")
                bch_sb = cpool.tile([ER, O_SZ], bf16, name="bch_sb")
                bcl_sb = cpool.tile([ER, O_SZ], bf16, name="bcl_sb")
                bias_sb = cpool.tile([128, E_SZ], f32, name="bias_sb")
                sel_sb = cpool.tile([E_SZ, ER], f32, name="sel_sb")
                ident_sb = cpool.tile([128, 128], f32, name="ident_sb")
                ident8_sb = cpool.tile([8, 8], f32, name="ident8_sb")
                hT_sb = cpool.tile([128, TPC], f32, name="hT_sb")
                ghh_sb = cpool.tile([128, NT, 128], bf16, name="ghh_sb")
                ghl_sb = cpool.tile([128, NT, 128], bf16, name="ghl_sb")

                # x hi/lo: 1MB per DMA, alternating HWDGE rings
                for g in range(NK // 4):
                    ksl = slice(g * 4, (g + 1) * 4)
                    eng = nc.sync if g % 2 == 0 else nc.scalar
                    eng.dma_start(
                        xh_sb[:, ksl, :], xt_d[0, ksl].rearrange("k p t -> p k t")
                    )
                    eng = nc.scalar if g % 2 == 0 else nc.sync
                    eng.dma_start(
                        xl_sb[:, ksl, :], xt_d[1, ksl].rearrange("k p t -> p k t")
                    )
                nc.scalar.dma_start(wrt_sb[:], wrt_d[:].rearrange("k p e -> p k e"))
                nc.scalar.dma_start(bch_sb[:], bcat_d[0])
                nc.scalar.dma_start(bcl_sb[:], bcat_d[1])
                nc.scalar.dma_start(bias_sb[:], bias_d[:])
                nc.scalar.dma_start(sel_sb[:], sel_d[:])
                make_identity(nc, ident_sb[:])
                make_identity(nc, ident8_sb[:])

                # ---- phase 1a: router logits (fp32) on streamed fp32 x ----
                do_router = "router" in parts
                do_h = "h" in parts
                do_gate = "gate" in parts and do_router and do_h
                do_p2 = "p2" in parts
                lgT_sb = cpool.tile([E_SZ, TPC], f32, name="lgT_sb")
                plT = [
                    psum.tile([E_SZ, 512], f32, tag="bank", name=f"plT{t}")
                    for t in range(TPC // 512)
                ]
                for kc in range(NK if do_router else 0):
                    xr_t = rpool.tile([128, TPC], f32, name="xr_t")
                    nc.gpsimd.dma_start(xr_t[:], xr_d[kc])
                    for tb in range(TPC // 512):
                        nc.tensor.matmul(
                            plT[tb][:],
                            wrt_sb[:, kc, :],
                            xr_t[:, tb * 512 : (tb + 1) * 512],
                            start=(kc == 0),
                            stop=(kc == NK - 1),
                        )
                if do_router:
                    for tb in range(TPC // 512):
                        nc.vector.tensor_copy(
                            lgT_sb[:, tb * 512 : (tb + 1) * 512], plT[tb][:]
                        )

                # ---- phase 1b: h^T = A_cat @ x^T  (bf16 x3), acat streamed ----
                ph = [
                    psum.tile([128, 512], f32, tag="bank", name=f"ph{t}")
                    for t in range(TPC // 512)
                ]
                for kc in range(NK if do_h else 0):
                    ac_t = acpool.tile([128, 2, ER], bf16, name="ac_t")
                    nc.scalar.dma_start(ac_t[:], acat_d[kc])
                    ach = ac_t[:, 0, :]
                    acl = ac_t[:, 1, :]
                    first = kc == 0
                    last = kc == NK - 1
                    for tb in range(TPC // 512):
                        sl = slice(tb * 512, (tb + 1) * 512)
                        nc.tensor.matmul(
                            ph[tb][:], ach, xh_sb[:, kc, sl], start=first, stop=False
                        )
                        nc.tensor.matmul(
                            ph[tb][:], ach, xl_sb[:, kc, sl], start=False, stop=False
                        )
                        nc.tensor.matmul(
                            ph[tb][:], acl, xh_sb[:, kc, sl], start=False, stop=last
                        )
                for tb in range(TPC // 512 if do_h else 0):
                    nc.vector.tensor_copy(
                        hT_sb[:, tb * 512 : (tb + 1) * 512], ph[tb][:]
                    )

                # ---- phase 1c: top-2 softmax gates + gh^T (hi/lo) ----
                if not do_gate:
                    nc.any.memset(ghh_sb[:], 0.0)
                    nc.any.memset(ghl_sb[:], 0.0)
                    if do_router:
                        dummy = gpool.tile([128, E_SZ], f32, name="dummy")
                        nc.vector.tensor_copy(dummy[:], lgT_sb[:, 0:E_SZ])
                    if do_h:
                        pass
                for tt in range(NT if do_gate else 0):
                    ts = slice(tt * 128, (tt + 1) * 128)
                    plg = psum.tile([128, E_SZ], f32, tag="bank", name=f"plg{tt}")
                    nc.tensor.transpose(plg[:], lgT_sb[:, ts], ident8_sb[:])
                    logit = gpool.tile([128, E_SZ], f32, name="logit")
                    nc.vector.tensor_tensor(
                        logit[:], plg[:], bias_sb[:], op=OP.add
                    )
                    m1 = gpool.tile([128, 1], f32, name="m1")
                    nc.vector.reduce_max(m1[:], logit[:], axis=AX)
                    mask1 = gpool.tile([128, E_SZ], f32, name="mask1")
                    nc.vector.tensor_scalar(
                        mask1[:], logit[:], m1[:], None, op0=OP.is_equal
                    )
                    l2 = gpool.tile([128, E_SZ], f32, name="l2")
                    nc.vector.tensor_scalar(l2[:], mask1[:], -1e30, None, op0=OP.mult)
                    nc.vector.tensor_tensor(l2[:], l2[:], logit[:], op=OP.add)
                    m2 = gpool.tile([128, 1], f32, name="m2")
                    nc.vector.reduce_max(m2[:], l2[:], axis=AX)
                    mask2 = gpool.tile([128, E_SZ], f32, name="mask2")
                    nc.vector.tensor_scalar(
                        mask2[:], l2[:], m2[:], None, op0=OP.is_equal
                    )
                    w1 = gpool.tile([128, 1], f32, name="w1")
                    nc.scalar.activation(
                        w1[:], m2[:], ACT.Sigmoid, bias=m1[:], scale=-1.0
                    )
                    w2 = gpool.tile([128, 1], f32, name="w2")
                    nc.vector.tensor_scalar(
                        w2[:], w1[:], -1.0, 1.0, op0=OP.mult, op1=OP.add
                    )
                    g1 = gpool.tile([128, E_SZ], f32, name="g1")
                    nc.vector.tensor_scalar(g1[:], mask1[:], w1[:], None, op0=OP.mult)
                    gates = gpool.tile([128, E_SZ], f32, name="gates")
                    nc.vector.tensor_scalar(
                        gates[:], mask2[:], w2[:], None, op0=OP.mult
                    )
                    nc.vector.tensor_tensor(gates[:], gates[:], g1[:], op=OP.add)

                    ptr = psum.tile([E_SZ, 128], f32, tag="bank", name=f"ptr{tt}")
                    nc.tensor.transpose(ptr[:], gates[:], ident_sb[:])
                    gT = gpool.tile([E_SZ, 128], f32, name="gT")
                    nc.vector.tensor_copy(gT[:], ptr[:])
                    pge = psum.tile([128, 128], f32, tag="bank", name=f"pge{tt}")
                    nc.tensor.matmul(pge[:], sel_sb[:], gT[:], start=True, stop=True)
                    gh32 = gpool.tile([128, 128], f32, name="gh32")
                    nc.vector.tensor_tensor(gh32[:], pge[:], hT_sb[:, ts], op=OP.mult)
                    nc.vector.tensor_copy(ghh_sb[:, tt, :], gh32[:])
                    glo = gpool.tile([128, 128], f32, name="glo")
                    nc.vector.tensor_tensor(
                        glo[:], gh32[:], ghh_sb[:, tt, :], op=OP.subtract
                    )
                    nc.vector.tensor_copy(ghl_sb[:, tt, :], glo[:])

                # ---- phase 2: out = x @ W^T (+ gh @ SCALING*B_cat), bf16 x3 ----
                for ot in range(NO if do_p2 else 0):
                    po = [
                        psum.tile([128, 512], f32, tag="bank", name=f"po{ot}_{i}")
                        for i in range(NT)
                    ]
                    for kcg in range(NKG):
                        w_t = wpool.tile([128, 2, 2, 512], bf16, name="w_t")
                        eng = nc.sync if kcg % 2 == 0 else nc.scalar
                        eng.dma_start(w_t[:], wt_d[ot, kcg])
                        for g in range(2):
                            kc = kcg * 2 + g
                            wh = w_t[:, g, 0, :]
                            wl = w_t[:, g, 1, :]
                            for tt in range(NT):
                                xh = xh_sb[:, kc, tt * 128 : (tt + 1) * 128]
                                xl = xl_sb[:, kc, tt * 128 : (tt + 1) * 128]
                                nc.tensor.matmul(
                                    po[tt][:], xh, wh, start=(kc == 0), stop=False
                                )
                                nc.tensor.matmul(
                                    po[tt][:], xh, wl, start=False, stop=False
                                )
                                nc.tensor.matmul(
                                    po[tt][:], xl, wh, start=False, stop=False
                                )
                    osl = slice(ot * 512, (ot + 1) * 512)
                    for tt in range(NT):
                        ghh = ghh_sb[:, tt, :]
                        ghl = ghl_sb[:, tt, :]
                        nc.tensor.matmul(
                            po[tt][:], ghh, bch_sb[:, osl], start=False, stop=False
                        )
                        nc.tensor.matmul(
                            po[tt][:], ghh, bcl_sb[:, osl], start=False, stop=False
                        )
                        nc.tensor.matmul(
                            po[tt][:], ghl, bch_sb[:, osl], start=False, stop=True
                        )
                        ob = opool.tile([128, 512], f32, name="ob")
                        nc.vector.tensor_copy(ob[:], po[tt][:])
                        eng = nc.gpsimd if tt % 2 == 0 else nc.scalar
                        eng.dma_start(out_d[ot, tt], ob[:])

            if loop_n is None:
                body()
            else:
                with tc.For_i(0, loop_n, 1) as iv:
                    body(iv)

    nc.compile()
    return nc


def _split_bf16(a):
    import ml_dtypes

    hi = a.astype(ml_dtypes.bfloat16)
    lo = (a - hi.astype(np.float32)).astype(ml_dtypes.bfloat16)
    return hi, lo


def _host_prep(x, language_ids, W, Wr, A, B):
    x = np.asarray(x, dtype=np.float32)
    W = np.asarray(W, dtype=np.float32)
    Wr = np.asarray(Wr, dtype=np.float32)
    A = np.asarray(A, dtype=np.float32)
    B = np.asarray(B, dtype=np.float32)
    lang = np.asarray(language_ids).astype(np.int64)

    xf = np.ascontiguousarray(x.reshape(B_SZ * T_SZ, D_SZ))

    # W^T [D,O]: hi/lo split, then [NO, NKG, 128, 2kc, 2hl, 512] row-contiguous
    wtT = W.T.reshape(NK, 128, NO, 512)                  # [kc, p, ot, n]
    wh, wl = _split_bf16(wtT)
    warr = np.stack([wh, wl], axis=-2)                   # [kc, p, ot, 2hl, n]
    warr = warr.reshape(NKG, 2, 128, NO, 2, 512)         # [kcg, g, p, ot, hl, n]
    wt = np.ascontiguousarray(warr.transpose(3, 0, 2, 1, 4, 5))

    acat_t = np.ascontiguousarray(A.reshape(ER, D_SZ).T).reshape(NK, 128, ER)
    ah, al = _split_bf16(acat_t)
    acat = np.ascontiguousarray(np.stack([ah, al], axis=2))  # [kc, p, 2, ER]

    wrt = np.ascontiguousarray(Wr.T).reshape(NK, 128, E_SZ)

    bcat32 = (SCALING * B.transpose(0, 2, 1)).reshape(ER, O_SZ)
    bh, bl = _split_bf16(bcat32)
    bcat = np.ascontiguousarray(np.stack([bh, bl], axis=0))  # [2, ER, O]

    sel = np.zeros((E_SZ, ER), dtype=np.float32)
    sel[np.arange(ER) // R_SZ, np.arange(ER)] = 1.0

    in_maps = []
    for c in range(NCORES):
        shard = xf[c * TPC : (c + 1) * TPC]
        xr = np.ascontiguousarray(shard.T).reshape(NK, 128, TPC)
        xhh, xll = _split_bf16(xr)
        xt = np.ascontiguousarray(np.stack([xhh, xll], axis=0))  # [2, kc, p, t]
        b = int(lang[(c * TPC) // T_SZ])
        brow = np.zeros(E_SZ, dtype=np.float32)
        if b >= 0:
            brow[b] = LANG_BIAS
        bias = np.ascontiguousarray(np.tile(brow, (128, 1)))
        in_maps.append(
            {
                "xt": xt,
                "xr": xr,
                "wt": wt,
                "acat": acat,
                "wrt": wrt,
                "bcat": bcat,
                "biasx": bias,
                "sel": sel,
            }
        )
    return in_maps


def kernel(x, language_ids, W, Wr, A, B):
    global LAST_RESULT
    from concourse.bass_utils import run_bass_kernel_spmd

    if "nc" not in _CACHE:
        _CACHE["nc"] = _build_bass()
    nc = _CACHE["nc"]

    in_maps = _host_prep(x, language_ids, W, Wr, A, B)
    res = run_bass_kernel_spmd(nc, in_maps, core_ids=list(range(NCORES)))
    LAST_RESULT = res
    outs = [
        r["out"].transpose(1, 2, 0, 3).reshape(TPC, O_SZ) for r in res.results
    ]
    return np.concatenate(outs, axis=0).reshape(B_SZ, T_SZ, O_SZ)
